# revision 1
# baseline (speedup 1.0000x reference)
"""2-layer GCN (GCNConv x2 + log_softmax) on 8 trn2 NeuronCores via Bass/Tile.

Math (identical to the reference by associativity + rank-1 factorization of
the symmetric normalization):
  dis = rsqrt(deg) with self-loops;  A_hat = D^-1/2 (A+I) D^-1/2
  L1: h1 = relu(dis * segsum(T1[src]) + b1),  T1 = dis * (x @ W1)   (16-wide)
  L2: y  = log_softmax((dis * segsum(T2[src])) @ W2 + b2), T2 = dis * h1
Both edge passes are gather+segment-sum of 16-wide rows with identical
structure (W2 is applied AFTER aggregation, so layer 2 also gathers 16-wide).

Sharding: nodes split into 8 contiguous ranges (dst-sharding); each core owns
the CSR rows of its range, degree-sorts its nodes so padded CSR tiles have
near-uniform width, and gathers from an AllGather'd full 16-wide table.
"""

import numpy as np

import concourse.bass as bass
import concourse.mybir as mybir
import concourse.tile as tile
from concourse.masks import make_identity
from concourse.vector_clock import ScopedClock

P = 128
F1 = 16
F2 = 40
D = 512
GROUP = 7
N_NODES = 100000
N_CORES = 8

# ---------------------------------------------------------------------------
# workaround: this walrus build rejects >1 sync wait per instruction and the
# Drain opcode; spill extra waits onto single-wait nops.
_nop_counter = [0]


def _fresh_nop(engine, wait):
    _nop_counter[0] += 1
    nop = mybir.InstNoOp(name=f"WSPILL-{_nop_counter[0]}", ins=[], outs=[])
    nop.engine = engine
    nop.sync_info = mybir.SyncInfo(on_wait=[wait], on_update=[])
    return nop


def _split_multi_waits(nc):
    for fn in nc.m.functions:
        for bb in fn.blocks:
            insts = bb.instructions
            if not any(
                i.sync_info is not None and len(i.sync_info.on_wait) > 1
                for i in insts
            ):
                continue
            newlist = []
            for inst in insts:
                si = inst.sync_info
                if si is not None and len(si.on_wait) > 1:
                    waits = list(si.on_wait)
                    for w in waits[:-1]:
                        newlist.append(_fresh_nop(inst.engine, w))
                    si.on_wait = waits[-1:]
                    inst.sync_info = si
                newlist.append(inst)
            insts[:] = newlist


def _patched_drain_and_barrier(self, tick_clock, wait_clock):
    nc = self.nc
    drain_inst = nc.sync.nop(nofuse=True, hint="tail_drain_nop")
    wait_clock.add_sem_waits(
        drain_inst.ins, ScopedClock({None: tick_clock.global_clock})
    )
    nc.all_engine_barrier()
    assert self.sems is not None
    popped = nc._tile_sem_poison_stack.pop()
    assert popped is self._sem_poison
    nc.clear_and_free_semaphores(list(self.sems.allocated().values()))
    nc.all_engine_barrier()


tile.TileContext._drain_and_barrier = _patched_drain_and_barrier


# ---------------------------------------------------------------------------
def _ceil_to(x, m):
    return (x + m - 1) // m * m


def _preprocess(edge_index, N, C):
    S = N // C
    T = (S + P - 1) // P
    TP = T * P
    PAD = N

    e = np.asarray(edge_index)
    src = np.concatenate([e[0], np.arange(N, dtype=e.dtype)]).astype(np.int64)
    dst = np.concatenate([e[1], np.arange(N, dtype=e.dtype)]).astype(np.int64)
    order = np.argsort(dst, kind="stable")
    srcs = src[order].astype(np.int32)
    dsts = dst[order]
    row_ptr = np.searchsorted(dsts, np.arange(N + 1)).astype(np.int64)
    deg = np.diff(row_ptr).astype(np.int32)

    perms, degqs = [], []
    Ks = np.zeros((C, T), np.int32)
    for c in range(C):
        lo = c * S
        deg_c = deg[lo : lo + S]
        perm = np.argsort(deg_c, kind="stable")
        perms.append(perm)
        degp = deg_c[perm]
        degq = np.ones(TP, np.float32)
        degq[:S] = degp
        degqs.append(degq)
        for t in range(T):
            seg = degp[t * P : (t + 1) * P]
            Ks[c, t] = _ceil_to(int(seg.max()) if len(seg) else 1, 4)
    K = Ks.max(axis=0)

    pos_map = np.empty(N + 1, np.int64)
    for c in range(C):
        lo = c * S
        pos_map[lo + perms[c]] = lo + np.arange(S)
    pos_map[N] = PAD

    E_tot = len(srcs)
    slots1, slots2 = [], []
    for c in range(C):
        lo = c * S
        perm = perms[c]
        parts = []
        for t in range(T):
            pn = perm[t * P : (t + 1) * P]
            nodes = lo + pn
            base = row_ptr[nodes]
            dg = deg[nodes]
            if len(pn) < P:
                base = np.concatenate([base, np.zeros(P - len(pn), np.int64)])
                dg = np.concatenate([dg, np.zeros(P - len(pn), np.int32)])
            kk = np.arange(K[t])
            mat = base[:, None] + kk[None, :]
            valid = kk[None, :] < dg[:, None]
            parts.append(
                np.where(valid, srcs[np.minimum(mat, E_tot - 1)], PAD).astype(np.int64)
            )
        flat = []
        for g0 in range(0, T, GROUP):
            flat.append(np.concatenate(parts[g0 : g0 + GROUP], axis=1).reshape(-1))
        s1 = np.concatenate(flat)
        slots1.append(s1.astype(np.int32))
        slots2.append(pos_map[s1].astype(np.int32))

    degn = np.ones((C, TP), np.float32)
    for c in range(C):
        degn[c, :S] = deg[c * S : (c + 1) * S]

    Kg = [int(K[g0 : g0 + GROUP].sum()) for g0 in range(0, T, GROUP)]
    meta = dict(
        N=N, C=C, S=S, T=T, TP=TP, NG=(T + GROUP - 1) // GROUP,
        K=[int(k) for k in K], Kg=Kg, TOT=int(slots1[0].size),
    )
    percore = dict(slots1=slots1, slots2=slots2, degq=degqs, degn=degn, perms=perms)
    return meta, percore


def _build_program(meta):
    N, C, S, T, TP = meta["N"], meta["C"], meta["S"], meta["T"], meta["TP"]
    K, Kg, NG, TOT = meta["K"], meta["Kg"], meta["NG"], meta["TOT"]
    fp = mybir.dt.float32

    nc = bass.Bass("TRN2", target_bir_lowering=False, debug=False, num_devices=C)
    x_in = nc.declare_dram_parameter("x", [S, D], fp, isOutput=False)
    w1_in = nc.declare_dram_parameter("W1", [D, F1], fp, isOutput=False)
    b1_in = nc.declare_dram_parameter("b1", [1, F1], fp, isOutput=False)
    w2_in = nc.declare_dram_parameter("W2", [F1, F2], fp, isOutput=False)
    b2_in = nc.declare_dram_parameter("b2", [1, F2], fp, isOutput=False)
    degn_in = nc.declare_dram_parameter("degn", [TP], fp, isOutput=False)
    degq_in = nc.declare_dram_parameter("degq", [TP], fp, isOutput=False)
    s1_in = nc.declare_dram_parameter("slots1", [TOT], mybir.dt.int32, isOutput=False)
    s2_in = nc.declare_dram_parameter("slots2", [TOT], mybir.dt.int32, isOutput=False)
    y_out = nc.declare_dram_parameter("y", [TP, F2], fp, isOutput=True)

    q_mine = nc.dram_tensor("q_mine", [S, F1], fp)
    u_mine = nc.dram_tensor("u_mine", [S, F1], fp)
    T1 = nc.dram_tensor("T1", [N + 1, F1], fp, addr_space="Shared")
    T2 = nc.dram_tensor("T2", [N + 1, F1], fp, addr_space="Shared")
    groups = [list(range(C))]

    with tile.TileContext(nc) as tc:
        with tc.tile_pool(name="const", bufs=1) as cpool, \
             tc.tile_pool(name="xp", bufs=3) as xp, \
             tc.tile_pool(name="xtp", bufs=3) as xtp, \
             tc.tile_pool(name="ps", bufs=2, space="PSUM") as ps, \
             tc.tile_pool(name="pt", bufs=2, space="PSUM") as pt, \
             tc.tile_pool(name="sl", bufs=2) as sl, \
             tc.tile_pool(name="gt", bufs=2) as gt, \
             tc.tile_pool(name="sm", bufs=6) as sm, \
             tc.tile_pool(name="ou", bufs=3) as ou:

            ident = cpool.tile([P, P], fp)
            make_identity(nc, ident[:])
            w1s = cpool.tile([P, (D // P) * F1], fp)
            nc.sync.dma_start(
                w1s[:].rearrange("p (k f) -> p k f", f=F1),
                w1_in.ap().rearrange("(k p) f -> p k f", p=P),
            )
            w2s = cpool.tile([F1, F2], fp)
            nc.sync.dma_start(w2s[:], w2_in[:, :])
            ones_row = cpool.tile([1, P], fp)
            nc.vector.memset(ones_row[:], 1.0)
            b1row = cpool.tile([1, F1], fp)
            nc.sync.dma_start(b1row[:], b1_in[:, :])
            b2row = cpool.tile([1, F2], fp)
            nc.sync.dma_start(b2row[:], b2_in[:, :])
            b1ps = pt.tile([P, F1], fp, space="PSUM", tag="hp")
            nc.tensor.matmul(b1ps[:], lhsT=ones_row[:], rhs=b1row[:], start=True, stop=True)
            b1t = cpool.tile([P, F1], fp)
            nc.vector.tensor_copy(b1t[:], b1ps[:])
            b2ps = pt.tile([P, F2], fp, space="PSUM", tag="wp")
            nc.tensor.matmul(b2ps[:], lhsT=ones_row[:], rhs=b2row[:], start=True, stop=True)
            b2t = cpool.tile([P, F2], fp)
            nc.vector.tensor_copy(b2t[:], b2ps[:])

            disn = cpool.tile([P, T], fp)
            nc.sync.dma_start(disn[:], degn_in.ap().rearrange("(t p) -> p t", p=P))
            nc.vector.reciprocal(disn[:], disn[:])
            nc.scalar.activation(disn[:], disn[:], mybir.ActivationFunctionType.Sqrt)
            disq = cpool.tile([P, T], fp)
            nc.sync.dma_start(disq[:], degq_in.ap().rearrange("(t p) -> p t", p=P))
            nc.vector.reciprocal(disq[:], disq[:])
            nc.scalar.activation(disq[:], disq[:], mybir.ActivationFunctionType.Sqrt)

            zrow = cpool.tile([1, F1], fp)
            nc.vector.memset(zrow[:], 0.0)
            nc.sync.dma_start(T1[N : N + 1, :], zrow[:])
            nc.sync.dma_start(T2[N : N + 1, :], zrow[:])

            # phase A: q = disn * (x @ W1)
            for t in range(T):
                rows = min(P, S - t * P)
                xt = xp.tile([P, D], fp, tag="xt")
                nc.sync.dma_start(xt[:rows, :], x_in[t * P : t * P + rows, :])
                hp = pt.tile([P, F1], fp, space="PSUM", tag="hp")
                for k in range(D // P):
                    tp_ = ps.tile([P, P], fp, space="PSUM", tag="tp")
                    nc.tensor.transpose(
                        tp_[:, :rows], xt[:rows, k * P : (k + 1) * P],
                        ident[:rows, :rows],
                    )
                    xts = xtp.tile([P, P], fp, tag="xts")
                    nc.vector.tensor_copy(xts[:, :rows], tp_[:, :rows])
                    nc.tensor.matmul(
                        hp[:rows, :], lhsT=xts[:, :rows],
                        rhs=w1s[:, k * F1 : (k + 1) * F1],
                        start=(k == 0), stop=(k == D // P - 1),
                    )
                qt = sm.tile([P, F1], fp, tag="qt")
                nc.vector.tensor_scalar(
                    qt[:rows, :], hp[:rows, :],
                    disn[:rows, t : t + 1], None, op0=mybir.AluOpType.mult,
                )
                nc.sync.dma_start(q_mine[t * P : t * P + rows, :], qt[:rows, :])

            nc.gpsimd.collective_compute(
                "AllGather", mybir.AluOpType.bypass, replica_groups=groups,
                ins=[q_mine[:, :]], outs=[T1[0:N, :]],
            )

            def agg_phase(table, slots_in, out_cb):
                off = 0
                for g in range(NG):
                    kg = Kg[g]
                    st = sl.tile([P, kg], mybir.dt.int32, tag="st")
                    nc.sync.dma_start(
                        st[:],
                        slots_in[off : off + P * kg].rearrange("(p k) -> p k", k=kg),
                    )
                    gtile = gt.tile([P, kg * F1], fp, tag="gt")
                    # one [P,1] indirect gather per slot column (the only
                    # offset shape this DGE unroll supports on HW)
                    for k in range(kg):
                        nc.gpsimd.indirect_dma_start(
                            out=gtile[:, k * F1 : (k + 1) * F1],
                            out_offset=None,
                            in_=table[:, :],
                            in_offset=bass.IndirectOffsetOnAxis(
                                ap=st[:, k : k + 1], axis=0
                            ),
                        )
                    ct = 0
                    for t in range(g * GROUP, min((g + 1) * GROUP, T)):
                        kt = K[t]
                        agg = sm.tile([P, F1], fp, tag="agg")
                        nc.vector.tensor_reduce(
                            out=agg[:, :, None],
                            in_=gtile[:, ct * F1 : (ct + kt) * F1].rearrange(
                                "p (k f) -> p f k", f=F1
                            ),
                            op=mybir.AluOpType.add, axis=mybir.AxisListType.X,
                        )
                        out_cb(t, agg)
                        ct += kt
                    off += P * kg

            def l1_out(t, agg):
                rows = min(P, S - t * P)
                tmp = sm.tile([P, F1], fp, tag="tmp1")
                nc.vector.tensor_scalar(
                    tmp[:], agg[:], disq[:, t : t + 1], None,
                    op0=mybir.AluOpType.mult,
                )
                nc.vector.tensor_add(tmp[:], tmp[:], b1t[:])
                ut = sm.tile([P, F1], fp, tag="ut")
                nc.vector.tensor_scalar(
                    ut[:], tmp[:], 0.0, disq[:, t : t + 1],
                    op0=mybir.AluOpType.max, op1=mybir.AluOpType.mult,
                )
                nc.sync.dma_start(u_mine[t * P : t * P + rows, :], ut[:rows, :])

            agg_phase(T1, s1_in, l1_out)

            nc.gpsimd.collective_compute(
                "AllGather", mybir.AluOpType.bypass, replica_groups=groups,
                ins=[u_mine[:, :]], outs=[T2[0:N, :]],
            )

            def l2_out(t, agg):
                rows = min(P, S - t * P)
                v = sm.tile([P, F1], fp, tag="v")
                nc.vector.tensor_scalar(
                    v[:], agg[:], disq[:, t : t + 1], None,
                    op0=mybir.AluOpType.mult,
                )
                vtp = ps.tile([P, P], fp, space="PSUM", tag="vtp")
                nc.tensor.transpose(vtp[:F1, :], v[:, :], ident[:])
                vts = sm.tile([F1, P], fp, tag="vts")
                nc.vector.tensor_copy(vts[:, :], vtp[:F1, :])
                wp = pt.tile([P, F2], fp, space="PSUM", tag="wp")
                nc.tensor.matmul(wp[:], lhsT=vts[:, :], rhs=w2s[:, :], start=True, stop=True)
                w = ou.tile([P, F2], fp, tag="w")
                nc.vector.tensor_add(w[:], wp[:], b2t[:])
                mx = sm.tile([P, 1], fp, tag="mx")
                nc.vector.tensor_reduce(
                    out=mx[:], in_=w[:], op=mybir.AluOpType.max,
                    axis=mybir.AxisListType.X,
                )
                nmx = sm.tile([P, 1], fp, tag="nmx")
                nc.vector.tensor_scalar_mul(nmx[:], mx[:], -1.0)
                ex = ou.tile([P, F2], fp, tag="ex")
                se = sm.tile([P, 1], fp, tag="se")
                nc.scalar.activation(
                    ex[:], w[:], mybir.ActivationFunctionType.Exp,
                    bias=nmx[:], accum_out=se[:],
                )
                ls = sm.tile([P, 1], fp, tag="ls")
                nc.scalar.activation(ls[:], se[:], mybir.ActivationFunctionType.Ln)
                yt = ou.tile([P, F2], fp, tag="yt")
                nc.vector.tensor_scalar(
                    yt[:], w[:], mx[:], ls[:],
                    op0=mybir.AluOpType.subtract, op1=mybir.AluOpType.subtract,
                )
                nc.sync.dma_start(y_out[t * P : t * P + rows, :], yt[:rows, :])

            agg_phase(T2, s2_in, l2_out)

    _split_multi_waits(nc)
    return nc


# ---------------------------------------------------------------------------
class _Runner:
    def __init__(self, nc, n_cores):
        import jax
        from jax.sharding import Mesh, PartitionSpec
        from jax.experimental.shard_map import shard_map
        from concourse.bass2jax import (
            _bass_exec_p, partition_id_tensor, install_neuronx_cc_hook,
        )

        install_neuronx_cc_hook()
        self.jax = jax
        self.n_cores = n_cores
        in_names, out_names, out_avals = [], [], []
        partition_name = (
            nc.partition_id_tensor.name if nc.partition_id_tensor else None
        )
        for alloc in nc.m.functions[0].allocations:
            if not isinstance(alloc, mybir.MemoryLocationSet):
                continue
            name = alloc.memorylocations[0].name
            if alloc.kind == "ExternalInput":
                if name != partition_name:
                    in_names.append(name)
            elif alloc.kind == "ExternalOutput":
                out_names.append(name)
                out_avals.append(
                    jax.core.ShapedArray(
                        tuple(alloc.tensor_shape), mybir.dt.np(alloc.dtype)
                    )
                )
        self.in_names, self.out_names, self.out_avals = in_names, out_names, out_avals
        n_params, n_outs = len(in_names), len(out_avals)
        all_in = in_names + out_names
        if partition_name is not None:
            all_in.append(partition_name)

        def _body(*args):
            operands = list(args)
            if partition_name is not None:
                operands.append(partition_id_tensor())
            return tuple(
                _bass_exec_p.bind(
                    *operands, out_avals=tuple(out_avals), in_names=tuple(all_in),
                    out_names=tuple(out_names), lowering_input_output_aliases=(),
                    sim_require_finite=True, sim_require_nnan=True, nc=nc,
                )
            )

        devices = jax.devices()[:n_cores]
        mesh = Mesh(np.asarray(devices), ("core",))
        self.fn = jax.jit(
            shard_map(
                _body, mesh=mesh,
                in_specs=(PartitionSpec("core"),) * (n_params + n_outs),
                out_specs=(PartitionSpec("core"),) * n_outs,
                check_rep=False,
            ),
            keep_unused=True,
        )

    def run(self, in_maps):
        concat = [
            np.concatenate([np.asarray(m[name]) for m in in_maps], axis=0)
            for name in self.in_names
        ]
        zeros = [
            np.zeros((self.n_cores * a.shape[0], *a.shape[1:]), a.dtype)
            for a in self.out_avals
        ]
        out = self.fn(*concat, *zeros)
        self.jax.block_until_ready(out)
        res = []
        for c in range(self.n_cores):
            res.append({
                name: np.asarray(out[i]).reshape(
                    self.n_cores, *self.out_avals[i].shape
                )[c]
                for i, name in enumerate(self.out_names)
            })
        return res


_CACHE = {}


def kernel(x, edge_index, W1, b1, W2, b2):
    x = np.asarray(x, np.float32)
    W1 = np.asarray(W1, np.float32)
    b1 = np.asarray(b1, np.float32)
    W2 = np.asarray(W2, np.float32)
    b2 = np.asarray(b2, np.float32)
    N, C = x.shape[0], N_CORES
    S = N // C

    meta, percore = _preprocess(edge_index, N, C)
    key = ("gcn", tuple(meta["K"]))
    if key not in _CACHE:
        nc = _build_program(meta)
        _CACHE[key] = _Runner(nc, C)
    runner = _CACHE[key]

    in_maps = []
    for c in range(C):
        in_maps.append({
            "x": x[c * S : (c + 1) * S],
            "W1": W1, "b1": b1[None], "W2": W2, "b2": b2[None],
            "degn": percore["degn"][c], "degq": percore["degq"][c],
            "slots1": percore["slots1"][c], "slots2": percore["slots2"][c],
        })
    res = runner.run(in_maps)

    y = np.empty((N, F2), np.float32)
    for c in range(C):
        y[c * S + percore["perms"][c]] = res[c]["y"][:S]
    return y

